# Initial kernel scaffold
#
"""Trainium2 Bass kernel for nn_Block_77481210020339 (HEALPix Swin-style block).

Pipeline: ff1(256->512)+gelu -> [LN -> win16-attn -> +res -> LN -> MLP(512->2048->512) -> +res] (block A)
          -> same shifted by ws/2=8 (block B) -> spectral-norm linear (512->256).

Sharding: 8 cores, each takes half of one batch image (24576 of 49152 pixels)
plus a redundantly-computed halo, so no collectives are needed.  Each core runs
two phases over 512-token slabs:
  phase 1: x -> ff1 -> block A -> h1 (HBM scratch, [c,t] layout)
  phase 2: h1 -> block B (shifted windows = aligned windows on slabs offset -8)
           -> ff2 (spectral-norm folded on host) -> out

Layout: activations are [channel-chunk(128 partitions), token] everywhere;
matmuls contract channels on partitions; window attention computes per-head
128x128 score tiles (8 windows of 16; off-block masked), softmax along the
free axis, PE-transpose of P, then PV with absolute-k contraction.
"""

import sys

sys.path.insert(0, "/opt/trn_rl_repo")

from contextlib import ExitStack

import numpy as np

import concourse.bass as bass
from concourse import bacc
import concourse.mybir as mybir
import concourse.tile as tile
from concourse.bass_utils import run_bass_kernel_spmd

F32 = mybir.dt.float32
F32R = mybir.dt.float32r
BF16 = mybir.dt.bfloat16
AF = mybir.ActivationFunctionType
ALU = mybir.AluOpType
GELU_FN = AF.Gelu  # test hook: CoreSim lacks Gelu; swap to Identity there
FF_DT = F32R  # ff1/ff2 matmul dtype (F32R = full speed; F32 = 4x slower, exact)
EN_LN = True    # debug knobs: replace stages with copies to bisect HW failures
EN_ATTN = True
EN_MLP = True
ATTN_DEPTH = 4  # 1=qkv/V/proj, 2=+scores/softmax, 3=+transpose, 4=full(PV)

# problem dims
B, N, CIN, CHID, COUT, WS, NH, HD = 4, 49152, 256, 512, 256, 16, 8, 64
CH4 = 4 * CHID  # 2048
P = 128
SLAB = 512
KC = CHID // P  # 4 channel chunks of the stream
TC = SLAB // P  # 4 token chunks per slab
HALO = 256


class Cfg:
    def __init__(self, t_out=24576, n_cores=8):
        self.t_out = t_out
        self.t_in = t_out + 2 * SLAB  # phase-1 scratch tokens
        self.n1 = self.t_in // SLAB  # phase-1 slabs
        self.n2 = t_out // SLAB + 1  # phase-2 slabs
        self.n_cores = n_cores


def _ceil_div(a, b):
    return (a + b - 1) // b


# ---------------------------------------------------------------------------
# program builder
# ---------------------------------------------------------------------------


def build_program(cfg: Cfg):
    nc = bacc.Bacc("TRN2", target_bir_lowering=False, debug=False,
                   enable_asserts=True, num_devices=cfg.n_cores)

    # ---- external params -------------------------------------------------
    x_in = nc.declare_dram_parameter("x_in", [cfg.t_in, CIN], F32, isOutput=False)
    wff1 = nc.declare_dram_parameter("wff1", [CIN // P, P, CHID], FF_DT, isOutput=False)
    bff1 = nc.declare_dram_parameter("bff1", [P, KC], F32, isOutput=False)
    wff2 = nc.declare_dram_parameter("wff2", [KC, P, COUT], FF_DT, isOutput=False)
    bff2row = nc.declare_dram_parameter("bff2row", [1, COUT], FF_DT, isOutput=False)

    blk = {}
    for p in ("a", "b"):
        blk[p] = dict(
            wqkv=nc.declare_dram_parameter(f"{p}_wqkv", [KC, P, 3 * CHID], BF16, isOutput=False),
            bqk=nc.declare_dram_parameter(f"{p}_bqk", [P, 2 * KC], F32, isOutput=False),
            bvrow=nc.declare_dram_parameter(f"{p}_bvrow", [1, CHID], BF16, isOutput=False),
            wpw=nc.declare_dram_parameter(f"{p}_wpw", [KC, P, CHID], BF16, isOutput=False),
            bpb=nc.declare_dram_parameter(f"{p}_bpb", [P, KC], F32, isOutput=False),
            wm1=nc.declare_dram_parameter(f"{p}_wm1", [KC, P, CH4], BF16, isOutput=False),
            bm1=nc.declare_dram_parameter(f"{p}_bm1", [P, CH4 // P], F32, isOutput=False),
            wm2=nc.declare_dram_parameter(f"{p}_wm2", [CH4 // P, P, CHID], BF16, isOutput=False),
            bm2=nc.declare_dram_parameter(f"{p}_bm2", [P, KC], F32, isOutput=False),
        )

    ident32 = nc.declare_dram_parameter("ident32", [P, P], F32, isOutput=False)
    maskbd = nc.declare_dram_parameter("maskbd", [P, P], F32, isOutput=False)
    ones_col_bf = nc.declare_dram_parameter("ones_col_bf", [P, 1], BF16, isOutput=False)
    ones_row_bf = nc.declare_dram_parameter("ones_row_bf", [1, P], BF16, isOutput=False)
    ones_row_f32 = nc.declare_dram_parameter("ones_row_f32", [1, P], FF_DT, isOutput=False)
    eps_t = nc.declare_dram_parameter("eps_t", [1, 1], F32, isOutput=False)

    out = nc.declare_dram_parameter("out", [cfg.t_out, COUT], F32, isOutput=True)

    ctx = ExitStack()
    with tile.TileContext(nc) as tc:
        with ctx:
            cpool = ctx.enter_context(tc.tile_pool(name="const", bufs=1))
            wpool = ctx.enter_context(tc.tile_pool(name="w", bufs=1))
            a1 = ctx.enter_context(tc.tile_pool(name="a1", bufs=1))
            a2 = ctx.enter_context(tc.tile_pool(name="a2", bufs=2))
            dpool = ctx.enter_context(tc.tile_pool(name="dram", bufs=1, space="DRAM"))
            ps2 = ctx.enter_context(tc.tile_pool(name="ps2", bufs=2, space="PSUM"))
            ps1 = ctx.enter_context(tc.tile_pool(name="ps1", bufs=1, space="PSUM"))

            # ---- constants ----
            ident_t = cpool.tile([P, P], F32, name="ident_t")
            nc.sync.dma_start(out=ident_t[:], in_=ident32[:])
            mask_t = cpool.tile([P, P], F32, name="mask_t")
            nc.sync.dma_start(out=mask_t[:], in_=maskbd[:])
            onescol_t = cpool.tile([P, 1], BF16, name="onescol_t")
            nc.sync.dma_start(out=onescol_t[:], in_=ones_col_bf[:])
            onesrow_t = cpool.tile([1, P], BF16, name="onesrow_t")
            nc.sync.dma_start(out=onesrow_t[:], in_=ones_row_bf[:])
            onesrowf_t = cpool.tile([1, P], FF_DT, name="onesrowf_t")
            nc.sync.dma_start(out=onesrowf_t[:], in_=ones_row_f32[:])
            epsc_t = cpool.tile([1, 1], F32, name="epsc_t")
            nc.sync.dma_start(out=epsc_t[:], in_=eps_t[:])
            wff1_t = cpool.tile([P, (CIN // P) * CHID], FF_DT, name="wff1_t")
            for j in range(CIN // P):
                nc.sync.dma_start(out=wff1_t[:, j * CHID:(j + 1) * CHID], in_=wff1[j])
            bff1_t = cpool.tile([P, KC], F32, name="bff1_t")
            nc.sync.dma_start(out=bff1_t[:], in_=bff1[:])
            wff2_t = cpool.tile([P, KC * COUT], FF_DT, name="wff2_t")
            for j in range(KC):
                nc.sync.dma_start(out=wff2_t[:, j * COUT:(j + 1) * COUT], in_=wff2[j])
            bff2_t = cpool.tile([1, COUT], FF_DT, name="bff2_t")
            nc.sync.dma_start(out=bff2_t[:], in_=bff2row[:])

            # ---- HBM scratch for block-A output ----
            h1_t = dpool.tile([KC, P, cfg.t_in], F32, name="h1_t")

            # ---------------------------------------------------------------
            def load_block_weights(p):
                w = blk[p]
                t = {}
                t["wqkv"] = [wpool.tile([P, 3 * CHID], BF16, name=f"wqkv{k}", tag=f"wqkv{k}") for k in range(KC)]
                for k in range(KC):
                    nc.sync.dma_start(out=t["wqkv"][k][:], in_=w["wqkv"][k])
                t["bqk"] = wpool.tile([P, 2 * KC], F32, name="bqk", tag="bqk")
                nc.sync.dma_start(out=t["bqk"][:], in_=w["bqk"][:])
                t["bvrow"] = wpool.tile([1, CHID], BF16, name="bvrow", tag="bvrow")
                nc.sync.dma_start(out=t["bvrow"][:], in_=w["bvrow"][:])
                t["wpw"] = [wpool.tile([P, CHID], BF16, name=f"wpw{k}", tag=f"wpw{k}") for k in range(KC)]
                for k in range(KC):
                    nc.sync.dma_start(out=t["wpw"][k][:], in_=w["wpw"][k])
                t["bpb"] = wpool.tile([P, KC], F32, name="bpb", tag="bpb")
                nc.sync.dma_start(out=t["bpb"][:], in_=w["bpb"][:])
                t["wm1"] = [wpool.tile([P, CH4], BF16, name=f"wm1{k}", tag=f"wm1{k}") for k in range(KC)]
                for k in range(KC):
                    nc.sync.dma_start(out=t["wm1"][k][:], in_=w["wm1"][k])
                t["bm1"] = wpool.tile([P, CH4 // P], F32, name="bm1", tag="bm1")
                nc.sync.dma_start(out=t["bm1"][:], in_=w["bm1"][:])
                t["wm2"] = [wpool.tile([P, CHID], BF16, name=f"wm2{k}", tag=f"wm2{k}") for k in range(CH4 // P)]
                for k in range(CH4 // P):
                    nc.sync.dma_start(out=t["wm2"][k][:], in_=w["wm2"][k])
                t["bm2"] = wpool.tile([P, KC], F32, name="bm2", tag="bm2")
                nc.sync.dma_start(out=t["bm2"][:], in_=w["bm2"][:])
                return t

            # ---------------------------------------------------------------
            def emit_ln(h_t, y_name, ypool):
                """h_t: [128, KC*512] f32 stream -> y bf16 (unit layernorm)."""
                hbf = a1.tile([P, KC * SLAB], BF16, name=f"{y_name}_hbf", tag="ln_hbf")
                sq = a1.tile([P, KC * SLAB], BF16, name=f"{y_name}_sq", tag="ln_sq")
                for k in range(KC):
                    sl = slice(k * SLAB, (k + 1) * SLAB)
                    nc.scalar.activation(hbf[:, sl], h_t[:, sl], AF.Copy)
                    nc.vector.tensor_mul(sq[:, sl], hbf[:, sl], hbf[:, sl])
                st = ps1.tile([64, SLAB], F32, name=f"{y_name}_st", tag="stats")
                for k in range(KC):
                    sl = slice(k * SLAB, (k + 1) * SLAB)
                    nc.tensor.matmul(st[0:1, :], onescol_t[:], hbf[:, sl],
                                     start=(k == 0), stop=(k == KC - 1))
                for k in range(KC):
                    sl = slice(k * SLAB, (k + 1) * SLAB)
                    nc.tensor.matmul(st[32:33, :], onescol_t[:], sq[:, sl],
                                     start=(k == 0), stop=(k == KC - 1))
                ms = a1.tile([1, SLAB], F32, name=f"{y_name}_ms", tag="ln_ms")
                nc.scalar.activation(ms[:], st[0:1, :], AF.Square, scale=1.0 / CHID)
                var = a1.tile([1, SLAB], F32, name=f"{y_name}_var", tag="ln_var")
                nc.vector.scalar_tensor_tensor(var[:], st[32:33, :], 1.0 / CHID, ms[:],
                                               op0=ALU.mult, op1=ALU.subtract)
                std = a1.tile([1, SLAB], F32, name=f"{y_name}_std", tag="ln_std")
                nc.scalar.activation(std[:], var[:], AF.Sqrt, bias=epsc_t[0:1, 0:1])
                r = a1.tile([1, SLAB], F32, name=f"{y_name}_r", tag="ln_r")
                nc.vector.reciprocal(r[:], std[:])
                rbf = a1.tile([1, SLAB], BF16, name=f"{y_name}_rbf", tag="ln_rbf")
                nc.scalar.activation(rbf[:], r[:], AF.Copy)
                mrbf = a1.tile([1, SLAB], BF16, name=f"{y_name}_mrbf", tag="ln_mrbf")
                nc.vector.scalar_tensor_tensor(mrbf[:], st[0:1, :], 1.0 / CHID, r[:],
                                               op0=ALU.mult, op1=ALU.mult)
                rb = ps2.tile([P, SLAB], F32, name=f"{y_name}_rb", tag="mmout")
                nc.tensor.matmul(rb[:], onesrow_t[:], rbf[:])
                mrb = ps2.tile([P, SLAB], F32, name=f"{y_name}_mrb", tag="mmout")
                nc.tensor.matmul(mrb[:], onesrow_t[:], mrbf[:])
                y = ypool.tile([P, KC * SLAB], BF16, name=y_name, tag=y_name)
                tmp = a1.tile([P, SLAB], F32, name=f"{y_name}_tmp", tag="ln_tmp")
                for k in range(KC):
                    sl = slice(k * SLAB, (k + 1) * SLAB)
                    nc.vector.tensor_mul(tmp[:], h_t[:, sl], rb[:])
                    nc.vector.tensor_sub(y[:, sl], tmp[:], mrb[:])
                return y

            # ---------------------------------------------------------------
            def emit_block(h_t, w, out_dtype=F32):
                """One transformer block on a 512-token slab.
                h_t: [128, KC*512] f32 (residual stream, [c,t] layout).
                Returns x2 tile (same layout)."""
                # ---- LN1 -> y ----
                if EN_LN:
                    y = emit_ln(h_t, "y_bf", a2)
                else:
                    y = a2.tile([P, KC * SLAB], BF16, name="y_bf", tag="y_bf")
                    for k in range(KC):
                        sl = slice(k * SLAB, (k + 1) * SLAB)
                        nc.scalar.activation(y[:, sl], h_t[:, sl], AF.Copy)

                if not EN_ATTN:
                    x1 = a1.tile([P, KC * SLAB], F32, name="x1", tag="x1")
                    for k in range(KC):
                        sl = slice(k * SLAB, (k + 1) * SLAB)
                        nc.vector.tensor_scalar(x1[:, sl], h_t[:, sl], 1.0, None,
                                                op0=ALU.mult)
                    return _finish_block(x1, h_t, w, out_dtype)
                # ---- qkv: Q^T,K^T (weight-stationary) ----
                # cols [0, 4096): m-chunk evictions (head pair 2m/2m+1 stacked);
                # cols [4096, 8192): bottom halves (odd heads) DMA-moved to
                # partitions 0-63 -- matmul operands at partition base 64 crash
                # the PE when bases alternate across matmuls, so every S-matmul
                # operand must start at partition 0.
                QK2 = 2 * KC * SLAB
                qk = a2.tile([P, 2 * QK2], BF16, name="qk", tag="qk", bufs=1)
                for m in range(2 * KC):
                    ps = ps2.tile([P, SLAB], F32, name=f"qkps{m}", tag="mmout")
                    for k in range(KC):
                        nc.tensor.matmul(ps[:], w["wqkv"][k][:, m * P:(m + 1) * P],
                                         y[:, k * SLAB:(k + 1) * SLAB],
                                         start=(k == 0), stop=(k == KC - 1))
                    nc.scalar.activation(qk[:, m * SLAB:(m + 1) * SLAB], ps[:],
                                         AF.Identity, bias=w["bqk"][:, m:m + 1])
                    nc.sync.dma_start(out=qk[0:64, QK2 + m * SLAB:QK2 + (m + 1) * SLAB],
                                      in_=qk[64:128, m * SLAB:(m + 1) * SLAB])

                # ---- V (activation-stationary -> [t, d]) ----
                v = a2.tile([P, KC * SLAB], BF16, name="v", tag="v")
                for tch in range(TC):
                    ps = ps2.tile([P, CHID], F32, name=f"vps{tch}", tag="mmout")
                    for k in range(KC):
                        nc.tensor.matmul(ps[:], y[:, k * SLAB + tch * P: k * SLAB + (tch + 1) * P],
                                         w["wqkv"][k][:, 2 * CHID:3 * CHID],
                                         start=(k == 0), stop=False)
                    nc.tensor.matmul(ps[:], onesrow_t[:], w["bvrow"][:],
                                     start=False, stop=True)
                    nc.scalar.activation(v[:, tch * SLAB:(tch + 1) * SLAB], ps[:], AF.Copy)

                # ---- windowed attention, per 128-token chunk ----
                ot = a2.tile([P, KC * SLAB], BF16, name="ot", tag="ot")
                for tch in range(TC):
                    if ATTN_DEPTH < 2:
                        nc.scalar.activation(ot[:, tch * SLAB:(tch + 1) * SLAB],
                                             v[:, tch * SLAB:(tch + 1) * SLAB], AF.Copy)
                        continue
                    s_ps = ps1.tile([P, NH * P], F32, name=f"s{tch}", tag="S")
                    for h in range(NH):
                        hoff = 0 if h % 2 == 0 else QK2
                        qsl = qk[0:64, hoff + (h // 2) * SLAB + tch * P:
                                 hoff + (h // 2) * SLAB + (tch + 1) * P]
                        ksl = qk[0:64, hoff + (KC + h // 2) * SLAB + tch * P:
                                 hoff + (KC + h // 2) * SLAB + (tch + 1) * P]
                        nc.tensor.matmul(s_ps[:, h * P:(h + 1) * P], qsl, ksl)
                    e_t = a2.tile([P, NH * P], F32, name=f"e{tch}", tag="E")
                    for h in range(NH):
                        nc.scalar.activation(e_t[:, h * P:(h + 1) * P],
                                             s_ps[:, h * P:(h + 1) * P], AF.Exp,
                                             scale=float(HD) ** -0.5)
                    em_t = e_t
                    sums = a1.tile([P, NH], F32, name=f"sums{tch}", tag="sums")
                    for h in range(NH):
                        nc.vector.scalar_tensor_tensor(em_t[:, h * P:(h + 1) * P],
                                                       e_t[:, h * P:(h + 1) * P], 1.0,
                                                       mask_t[:],
                                                       op0=ALU.mult, op1=ALU.mult,
                                                       accum_out=sums[:, h:h + 1])
                    rec = a1.tile([P, NH], F32, name=f"rec{tch}", tag="rec")
                    nc.vector.reciprocal(rec[:], sums[:])
                    for h in range(NH):
                        nc.vector.tensor_scalar(em_t[:, h * P:(h + 1) * P],
                                                em_t[:, h * P:(h + 1) * P],
                                                rec[:, h:h + 1], None, op0=ALU.mult)
                    if ATTN_DEPTH < 3:
                        nc.scalar.activation(ot[:, tch * SLAB:(tch + 1) * SLAB],
                                             v[:, tch * SLAB:(tch + 1) * SLAB], AF.Copy)
                        continue
                    pt_ps = ps1.tile([P, NH * P], F32, name=f"ptps{tch}", tag="PT")
                    pt = a2.tile([P, NH * P], BF16, name=f"pt{tch}", tag="PTS")
                    for h in range(NH):
                        nc.tensor.transpose(pt_ps[:, h * P:(h + 1) * P],
                                            em_t[:, h * P:(h + 1) * P], ident_t[:])
                        nc.scalar.activation(pt[:, h * P:(h + 1) * P],
                                             pt_ps[:, h * P:(h + 1) * P], AF.Copy)
                    if ATTN_DEPTH < 4:
                        nc.scalar.activation(ot[:, tch * SLAB:(tch + 1) * SLAB],
                                             v[:, tch * SLAB:(tch + 1) * SLAB], AF.Copy)
                        continue
                    o_ps = ps1.tile([P, SLAB], F32, name=f"ops{tch}", tag="OT")
                    for h in range(NH):
                        nc.tensor.matmul(o_ps[64 * (h % 2):64 * (h % 2) + 64,
                                              (h // 2) * P:(h // 2 + 1) * P],
                                         v[:, tch * SLAB + 64 * h:tch * SLAB + 64 * h + 64],
                                         pt[:, h * P:(h + 1) * P])
                    nc.scalar.activation(ot[:, tch * SLAB:(tch + 1) * SLAB], o_ps[:], AF.Copy)

                # ---- proj + residual ----
                x1 = a1.tile([P, KC * SLAB], F32, name="x1", tag="x1")
                ot_r = ot[:].rearrange("p (t j q) -> p t j q", t=TC, j=KC, q=P)
                for m in range(KC):
                    ps = ps2.tile([P, SLAB], F32, name=f"pps{m}", tag="mmout")
                    for k in range(KC):
                        nc.tensor.matmul(ps[:], w["wpw"][k][:, m * P:(m + 1) * P],
                                         ot_r[:, :, k, :],
                                         start=(k == 0), stop=(k == KC - 1))
                    nc.vector.scalar_tensor_tensor(x1[:, m * SLAB:(m + 1) * SLAB], ps[:],
                                                   w["bpb"][:, m:m + 1],
                                                   h_t[:, m * SLAB:(m + 1) * SLAB],
                                                   op0=ALU.add, op1=ALU.add)
                return _finish_block(x1, h_t, w, out_dtype)

            def _finish_block(x1, h_t, w, out_dtype):
                # ---- LN2 -> z ----
                if EN_LN:
                    z = emit_ln(x1, "z_bf", a1)
                else:
                    z = a1.tile([P, KC * SLAB], BF16, name="z_bf", tag="z_bf")
                    for k in range(KC):
                        sl = slice(k * SLAB, (k + 1) * SLAB)
                        nc.scalar.activation(z[:, sl], x1[:, sl], AF.Copy)
                if not EN_MLP:
                    x2 = a2.tile([P, KC * SLAB], out_dtype, name="x2", tag="x2")
                    for k in range(KC):
                        sl = slice(k * SLAB, (k + 1) * SLAB)
                        nc.vector.tensor_scalar(x2[:, sl], x1[:, sl], 1.0, None,
                                                op0=ALU.mult)
                    return x2
                # ---- MLP ----
                g = a1.tile([P, (CH4 // P) * SLAB], BF16, name="g", tag="g")
                for m in range(CH4 // P):
                    ps = ps2.tile([P, SLAB], F32, name=f"m1ps{m}", tag="mmout")
                    for k in range(KC):
                        nc.tensor.matmul(ps[:], w["wm1"][k][:, m * P:(m + 1) * P],
                                         z[:, k * SLAB:(k + 1) * SLAB],
                                         start=(k == 0), stop=(k == KC - 1))
                    nc.scalar.activation(g[:, m * SLAB:(m + 1) * SLAB], ps[:], GELU_FN,
                                         bias=w["bm1"][:, m:m + 1])
                x2 = a2.tile([P, KC * SLAB], out_dtype, name="x2", tag="x2")
                for m in range(KC):
                    ps = ps2.tile([P, SLAB], F32, name=f"m2ps{m}", tag="mmout")
                    for k in range(CH4 // P):
                        nc.tensor.matmul(ps[:], w["wm2"][k][:, m * P:(m + 1) * P],
                                         g[:, k * SLAB:(k + 1) * SLAB],
                                         start=(k == 0), stop=(k == CH4 // P - 1))
                    nc.vector.scalar_tensor_tensor(x2[:, m * SLAB:(m + 1) * SLAB], ps[:],
                                                   w["bm2"][:, m:m + 1],
                                                   x1[:, m * SLAB:(m + 1) * SLAB],
                                                   op0=ALU.add, op1=ALU.add)
                return x2

            # ===============================================================
            # phase 1: x -> ff1 -> block A -> h1 scratch
            # ===============================================================
            t_w = load_block_weights("a")
            for b in range(cfg.n1):
                t0 = b * SLAB
                xin = a1.tile([P, TC * CIN], F32, name=f"xin{b}", tag="xin")
                for i in range(TC):
                    nc.sync.dma_start(out=xin[:, i * CIN:(i + 1) * CIN],
                                      in_=x_in[t0 + i * P:t0 + (i + 1) * P, :])
                xT = a1.tile([P, (CIN // P) * SLAB], FF_DT, name=f"xT{b}", tag="xT")
                for j in range(CIN // P):
                    tps = ps2.tile([P, SLAB], F32, name=f"tps{b}_{j}", tag="mmout")
                    for i in range(TC):
                        nc.tensor.transpose(tps[:, i * P:(i + 1) * P],
                                            xin[:, i * CIN + j * P:i * CIN + (j + 1) * P],
                                            ident_t[:])
                    nc.scalar.activation(xT[:, j * SLAB:(j + 1) * SLAB], tps[:], AF.Copy)
                h_t = a2.tile([P, KC * SLAB], F32, name=f"h{b}", tag="h")
                for m in range(KC):
                    ps = ps2.tile([P, SLAB], F32, name=f"f1ps{b}_{m}", tag="mmout")
                    for j in range(CIN // P):
                        nc.tensor.matmul(ps[:],
                                         wff1_t[:, j * CHID + m * P:j * CHID + (m + 1) * P],
                                         xT[:, j * SLAB:(j + 1) * SLAB],
                                         start=(j == 0), stop=(j == CIN // P - 1))
                    nc.scalar.activation(h_t[:, m * SLAB:(m + 1) * SLAB], ps[:], GELU_FN,
                                         bias=bff1_t[:, m:m + 1])
                x2 = emit_block(h_t, t_w)
                for k in range(KC):
                    nc.sync.dma_start(out=h1_t[k, :, t0:t0 + SLAB],
                                      in_=x2[:, k * SLAB:(k + 1) * SLAB])

            # ===============================================================
            # phase 2: h1 -> block B -> ff2 -> out
            # ===============================================================
            t_w = load_block_weights("b")
            for b in range(cfg.n2):
                c0 = b * SLAB + HALO - WS // 2  # slab origin in scratch coords
                h_t = a2.tile([P, KC * SLAB], F32, name=f"hb{b}", tag="h")
                for k in range(KC):
                    nc.sync.dma_start(out=h_t[:, k * SLAB:(k + 1) * SLAB],
                                      in_=h1_t[k, :, c0:c0 + SLAB])
                x2 = emit_block(h_t, t_w, out_dtype=FF_DT)
                # ff2 (fp32r, activation-stationary -> [t, c] directly)
                o_t = a1.tile([P, TC * COUT], F32, name=f"o{b}", tag="o")
                out_base = b * SLAB - WS // 2  # first out row this slab covers
                for tch in range(TC):
                    r0 = out_base + tch * P  # out rows [r0, r0+128)
                    lo, hi = max(r0, 0), min(r0 + P, cfg.t_out)
                    if lo >= hi:
                        continue
                    ps = ps2.tile([P, COUT], F32, name=f"f2ps{b}_{tch}", tag="mmout")
                    for k in range(KC):
                        nc.tensor.matmul(ps[:],
                                         x2[:, k * SLAB + tch * P:k * SLAB + (tch + 1) * P],
                                         wff2_t[:, k * COUT:(k + 1) * COUT],
                                         start=(k == 0), stop=False)
                    nc.tensor.matmul(ps[:], onesrowf_t[:],
                                     bff2_t[:], start=False, stop=True)
                    nc.scalar.activation(o_t[:, tch * COUT:(tch + 1) * COUT], ps[:], AF.Copy)
                    nc.sync.dma_start(out=out[lo:hi, :],
                                      in_=o_t[lo - r0:hi - r0, tch * COUT:(tch + 1) * COUT])

    nc.compile()
    return nc


# ---------------------------------------------------------------------------
# host-side input preparation
# ---------------------------------------------------------------------------


def _sigma(W, u):
    W = np.asarray(W, np.float32)
    u = np.asarray(u, np.float32)
    v = W @ u
    v = v / (np.linalg.norm(v) + 1e-12)
    u2 = v @ W
    u2 = u2 / (np.linalg.norm(u2) + 1e-12)
    return float(v @ W @ u2)


def prep_weights(inputs):
    """Host-side: fold LN affine + spectral norm into weights; tile/cast."""
    f32 = np.float32
    d = {}
    w1 = np.asarray(inputs["ff1_w"], f32)
    d["wff1"] = np.ascontiguousarray(w1.reshape(CIN // P, P, CHID))
    d["bff1"] = np.ascontiguousarray(np.asarray(inputs["ff1_b"], f32).reshape(KC, P).T)

    sig = _sigma(inputs["ff2_w"], inputs["ff2_u"])
    w2 = np.asarray(inputs["ff2_w"], f32) / sig
    d["wff2"] = np.ascontiguousarray(w2.reshape(KC, P, COUT))
    d["bff2row"] = np.asarray(inputs["ff2_b"], f32).reshape(1, COUT)

    for p in ("a", "b"):
        g1 = np.asarray(inputs[f"{p}_ln1g"], f32)
        b1 = np.asarray(inputs[f"{p}_ln1b"], f32)
        qkvw = np.asarray(inputs[f"{p}_qkvw"], f32)
        qkvb = np.asarray(inputs[f"{p}_qkvb"], f32) + b1 @ qkvw
        wg = g1[:, None] * qkvw  # [512, 1536]
        d[f"{p}_wqkv"] = _to_bf16(wg.reshape(KC, P, 3 * CHID))
        d[f"{p}_bqk"] = np.ascontiguousarray(qkvb[:2 * CHID].reshape(2 * KC, P).T)
        d[f"{p}_bvrow"] = _to_bf16(qkvb[2 * CHID:].reshape(1, CHID))
        pw = np.asarray(inputs[f"{p}_pw"], f32)
        d[f"{p}_wpw"] = _to_bf16(pw.reshape(KC, P, CHID))
        d[f"{p}_bpb"] = np.ascontiguousarray(np.asarray(inputs[f"{p}_pb"], f32).reshape(KC, P).T)
        g2 = np.asarray(inputs[f"{p}_ln2g"], f32)
        b2 = np.asarray(inputs[f"{p}_ln2b"], f32)
        m1w = np.asarray(inputs[f"{p}_m1w"], f32)
        m1b = np.asarray(inputs[f"{p}_m1b"], f32) + b2 @ m1w
        d[f"{p}_wm1"] = _to_bf16((g2[:, None] * m1w).reshape(KC, P, CH4))
        d[f"{p}_bm1"] = np.ascontiguousarray(m1b.reshape(CH4 // P, P).T)
        m2w = np.asarray(inputs[f"{p}_m2w"], f32)
        d[f"{p}_wm2"] = _to_bf16(m2w.reshape(CH4 // P, P, CHID))
        d[f"{p}_bm2"] = np.ascontiguousarray(np.asarray(inputs[f"{p}_m2b"], f32).reshape(KC, P).T)

    d["ident32"] = np.eye(P, dtype=f32)
    m = np.zeros((P, P), f32)
    for wdw in range(P // WS):
        m[wdw * WS:(wdw + 1) * WS, wdw * WS:(wdw + 1) * WS] = 1.0
    d["maskbd"] = m
    d["ones_col_bf"] = _to_bf16(np.ones((P, 1), f32))
    d["ones_row_bf"] = _to_bf16(np.ones((1, P), f32))
    d["ones_row_f32"] = np.ones((1, P), f32)
    d["eps_t"] = np.full((1, 1), 1e-5, f32)
    return d


def _to_bf16(a):
    import ml_dtypes
    return np.ascontiguousarray(np.asarray(a, np.float32)).astype(ml_dtypes.bfloat16)


def make_in_maps(x, wd, cfg: Cfg):
    """x: [B, N, CIN]. Returns per-core input maps."""
    maps = []
    cores_per_batch = max(1, cfg.n_cores // x.shape[0])
    for c in range(cfg.n_cores):
        beta = c // cores_per_batch
        eta = c % cores_per_batch
        start = eta * cfg.t_out - HALO
        idx = (start + np.arange(cfg.t_in)) % x.shape[1]
        m = {"x_in": np.ascontiguousarray(x[beta, idx], np.float32)}
        m.update(wd)
        maps.append(m)
    return maps


_PROG = {}


def _get_prog(cfg: Cfg):
    key = (cfg.t_out, cfg.n_cores)
    if key not in _PROG:
        _PROG[key] = build_program(cfg)
    return _PROG[key]


def kernel(**inputs) -> np.ndarray:
    x = np.asarray(inputs["x"], np.float32)
    Bx, Nx = x.shape[0], x.shape[1]
    n_cores = 8
    cores_per_batch = n_cores // Bx
    cfg = Cfg(t_out=Nx // cores_per_batch, n_cores=n_cores)
    nc = _get_prog(cfg)
    wd = prep_weights(inputs)
    in_maps = make_in_maps(x, wd, cfg)
    res = run_bass_kernel_spmd(nc, in_maps, core_ids=list(range(n_cores)))
    out = np.empty((Bx, Nx, COUT), np.float32)
    for c in range(n_cores):
        beta = c // cores_per_batch
        eta = c % cores_per_batch
        out[beta, eta * cfg.t_out:(eta + 1) * cfg.t_out] = res.results[c]["out"]
    return out



# revision 1
# speedup vs baseline: 18.6013x; 18.6013x over previous
"""Trainium2 Bass kernel for nn_Block_77481210020339 (HEALPix Swin-style block).

Pipeline: ff1(256->512)+gelu -> [LN -> win16-attn -> +res -> LN -> MLP(512->2048->512) -> +res] (block A)
          -> same shifted by ws/2=8 (block B) -> spectral-norm linear (512->256).

Sharding: 8 cores, each takes half of one batch image (24576 of 49152 pixels)
plus a redundantly-computed halo, so no collectives are needed.  Each core runs
two phases over 512-token slabs:
  phase 1: x -> ff1 -> block A -> h1 (HBM scratch, [c,t] layout)
  phase 2: h1 -> block B (shifted windows = aligned windows on slabs offset -8)
           -> ff2 (spectral-norm folded on host) -> out

Layout: activations are [channel-chunk(128 partitions), token] everywhere;
matmuls contract channels on partitions; window attention computes per-head
128x128 score tiles (8 windows of 16; off-block masked), softmax along the
free axis, PE-transpose of P, then PV with absolute-k contraction.
"""

import sys

sys.path.insert(0, "/opt/trn_rl_repo")

from contextlib import ExitStack

import numpy as np

import concourse.bass as bass
from concourse import bacc
import concourse.mybir as mybir
import concourse.tile as tile
from concourse.bass_utils import run_bass_kernel_spmd

F32 = mybir.dt.float32
F32R = mybir.dt.float32r
BF16 = mybir.dt.bfloat16
AF = mybir.ActivationFunctionType
ALU = mybir.AluOpType
GELU_FN = AF.Gelu  # test hook: CoreSim lacks Gelu; swap to Identity there
FF_DT = F32R  # ff1/ff2 matmul dtype (F32R = full speed; F32 = 4x slower, exact)
EN_LN = True    # debug knobs: replace stages with copies to bisect HW failures
EN_ATTN = True
EN_MLP = True
ATTN_DEPTH = 4  # 1=qkv/V/proj, 2=+scores/softmax, 3=+transpose, 4=full(PV)

# problem dims
B, N, CIN, CHID, COUT, WS, NH, HD = 4, 49152, 256, 512, 256, 16, 8, 64
CH4 = 4 * CHID  # 2048
P = 128
SLAB = 512
KC = CHID // P  # 4 channel chunks of the stream
TC = SLAB // P  # 4 token chunks per slab
HALO = 256


class Cfg:
    def __init__(self, t_out=24576, n_cores=8):
        self.t_out = t_out
        self.t_in = t_out + 2 * SLAB  # phase-1 scratch tokens
        self.n1 = self.t_in // SLAB  # phase-1 slabs
        self.n2 = t_out // SLAB + 1  # phase-2 slabs
        self.n_cores = n_cores


def _ceil_div(a, b):
    return (a + b - 1) // b


# ---------------------------------------------------------------------------
# program builder
# ---------------------------------------------------------------------------


def build_program(cfg: Cfg):
    nc = bacc.Bacc("TRN2", target_bir_lowering=False, debug=False,
                   enable_asserts=True, num_devices=cfg.n_cores)

    # ---- external params -------------------------------------------------
    x_in = nc.declare_dram_parameter("x_in", [cfg.t_in, CIN], F32, isOutput=False)
    wff1 = nc.declare_dram_parameter("wff1", [CIN // P, P, CHID], FF_DT, isOutput=False)
    bff1 = nc.declare_dram_parameter("bff1", [P, KC], F32, isOutput=False)
    wff2 = nc.declare_dram_parameter("wff2", [KC, P, COUT], FF_DT, isOutput=False)
    bff2row = nc.declare_dram_parameter("bff2row", [1, COUT], FF_DT, isOutput=False)

    blk = {}
    for p in ("a", "b"):
        blk[p] = dict(
            wqkv=nc.declare_dram_parameter(f"{p}_wqkv", [KC, P, 3 * CHID], BF16, isOutput=False),
            bqk=nc.declare_dram_parameter(f"{p}_bqk", [P, 2 * KC], F32, isOutput=False),
            bvrow=nc.declare_dram_parameter(f"{p}_bvrow", [1, CHID], BF16, isOutput=False),
            wpw=nc.declare_dram_parameter(f"{p}_wpw", [KC, P, CHID], BF16, isOutput=False),
            bpb=nc.declare_dram_parameter(f"{p}_bpb", [P, KC], F32, isOutput=False),
            wm1=nc.declare_dram_parameter(f"{p}_wm1", [KC, P, CH4], BF16, isOutput=False),
            bm1=nc.declare_dram_parameter(f"{p}_bm1", [P, CH4 // P], F32, isOutput=False),
            wm2=nc.declare_dram_parameter(f"{p}_wm2", [CH4 // P, P, CHID], BF16, isOutput=False),
            bm2=nc.declare_dram_parameter(f"{p}_bm2", [P, KC], F32, isOutput=False),
        )

    ident32 = nc.declare_dram_parameter("ident32", [P, P], F32, isOutput=False)
    maskbd = nc.declare_dram_parameter("maskbd", [P, P], F32, isOutput=False)
    ones_col_bf = nc.declare_dram_parameter("ones_col_bf", [P, 1], BF16, isOutput=False)
    ones_row_bf = nc.declare_dram_parameter("ones_row_bf", [1, P], BF16, isOutput=False)
    ones_row_f32 = nc.declare_dram_parameter("ones_row_f32", [1, P], FF_DT, isOutput=False)
    eps_t = nc.declare_dram_parameter("eps_t", [1, 1], F32, isOutput=False)

    out = nc.declare_dram_parameter("out", [cfg.t_out, COUT], F32, isOutput=True)

    ctx = ExitStack()
    with tile.TileContext(nc) as tc:
        with ctx:
            cpool = ctx.enter_context(tc.tile_pool(name="const", bufs=1))
            wpool = ctx.enter_context(tc.tile_pool(name="w", bufs=1))
            a1 = ctx.enter_context(tc.tile_pool(name="a1", bufs=1))
            a2 = ctx.enter_context(tc.tile_pool(name="a2", bufs=2))
            dpool = ctx.enter_context(tc.tile_pool(name="dram", bufs=1, space="DRAM"))
            ps2 = ctx.enter_context(tc.tile_pool(name="ps2", bufs=2, space="PSUM"))
            ps1 = ctx.enter_context(tc.tile_pool(name="ps1", bufs=1, space="PSUM"))

            # ---- constants ----
            ident_t = cpool.tile([P, P], F32, name="ident_t")
            nc.sync.dma_start(out=ident_t[:], in_=ident32[:])
            mask_t = cpool.tile([P, P], F32, name="mask_t")
            nc.sync.dma_start(out=mask_t[:], in_=maskbd[:])
            onescol_t = cpool.tile([P, 1], BF16, name="onescol_t")
            nc.sync.dma_start(out=onescol_t[:], in_=ones_col_bf[:])
            onesrow_t = cpool.tile([1, P], BF16, name="onesrow_t")
            nc.sync.dma_start(out=onesrow_t[:], in_=ones_row_bf[:])
            onesrowf_t = cpool.tile([1, P], FF_DT, name="onesrowf_t")
            nc.sync.dma_start(out=onesrowf_t[:], in_=ones_row_f32[:])
            epsc_t = cpool.tile([1, 1], F32, name="epsc_t")
            nc.sync.dma_start(out=epsc_t[:], in_=eps_t[:])
            wff1_t = cpool.tile([P, (CIN // P) * CHID], FF_DT, name="wff1_t")
            for j in range(CIN // P):
                nc.sync.dma_start(out=wff1_t[:, j * CHID:(j + 1) * CHID], in_=wff1[j])
            bff1_t = cpool.tile([P, KC], F32, name="bff1_t")
            nc.sync.dma_start(out=bff1_t[:], in_=bff1[:])
            wff2_t = cpool.tile([P, KC * COUT], FF_DT, name="wff2_t")
            for j in range(KC):
                nc.sync.dma_start(out=wff2_t[:, j * COUT:(j + 1) * COUT], in_=wff2[j])
            bff2_t = cpool.tile([1, COUT], FF_DT, name="bff2_t")
            nc.sync.dma_start(out=bff2_t[:], in_=bff2row[:])

            # ---- HBM scratch for block-A output ----
            h1_t = dpool.tile([KC, P, cfg.t_in], F32, name="h1_t")

            # ---------------------------------------------------------------
            def load_block_weights(p):
                w = blk[p]
                t = {}
                t["wqkv"] = [wpool.tile([P, 3 * CHID], BF16, name=f"wqkv{k}", tag=f"wqkv{k}") for k in range(KC)]
                for k in range(KC):
                    nc.sync.dma_start(out=t["wqkv"][k][:], in_=w["wqkv"][k])
                t["bqk"] = wpool.tile([P, 2 * KC], F32, name="bqk", tag="bqk")
                nc.sync.dma_start(out=t["bqk"][:], in_=w["bqk"][:])
                t["bvrow"] = wpool.tile([1, CHID], BF16, name="bvrow", tag="bvrow")
                nc.sync.dma_start(out=t["bvrow"][:], in_=w["bvrow"][:])
                t["wpw"] = [wpool.tile([P, CHID], BF16, name=f"wpw{k}", tag=f"wpw{k}") for k in range(KC)]
                for k in range(KC):
                    nc.sync.dma_start(out=t["wpw"][k][:], in_=w["wpw"][k])
                t["bpb"] = wpool.tile([P, KC], F32, name="bpb", tag="bpb")
                nc.sync.dma_start(out=t["bpb"][:], in_=w["bpb"][:])
                t["wm1"] = [wpool.tile([P, CH4], BF16, name=f"wm1{k}", tag=f"wm1{k}") for k in range(KC)]
                for k in range(KC):
                    nc.sync.dma_start(out=t["wm1"][k][:], in_=w["wm1"][k])
                t["bm1"] = wpool.tile([P, CH4 // P], F32, name="bm1", tag="bm1")
                nc.sync.dma_start(out=t["bm1"][:], in_=w["bm1"][:])
                t["wm2"] = [wpool.tile([P, CHID], BF16, name=f"wm2{k}", tag=f"wm2{k}") for k in range(CH4 // P)]
                for k in range(CH4 // P):
                    nc.sync.dma_start(out=t["wm2"][k][:], in_=w["wm2"][k])
                t["bm2"] = wpool.tile([P, KC], F32, name="bm2", tag="bm2")
                nc.sync.dma_start(out=t["bm2"][:], in_=w["bm2"][:])
                return t

            # ---------------------------------------------------------------
            def emit_ln(h_t, y_name, ypool):
                """h_t: [128, KC*512] f32 stream -> y bf16 (unit layernorm)."""
                hbf = a1.tile([P, KC * SLAB], BF16, name=f"{y_name}_hbf", tag="ln_hbf")
                sq = a1.tile([P, KC * SLAB], BF16, name=f"{y_name}_sq", tag="ln_sq")
                for k in range(KC):
                    sl = slice(k * SLAB, (k + 1) * SLAB)
                    nc.scalar.activation(hbf[:, sl], h_t[:, sl], AF.Copy)
                    nc.vector.tensor_mul(sq[:, sl], hbf[:, sl], hbf[:, sl])
                st = ps1.tile([64, SLAB], F32, name=f"{y_name}_st", tag="stats")
                for k in range(KC):
                    sl = slice(k * SLAB, (k + 1) * SLAB)
                    nc.tensor.matmul(st[0:1, :], onescol_t[:], hbf[:, sl],
                                     start=(k == 0), stop=(k == KC - 1))
                for k in range(KC):
                    sl = slice(k * SLAB, (k + 1) * SLAB)
                    nc.tensor.matmul(st[32:33, :], onescol_t[:], sq[:, sl],
                                     start=(k == 0), stop=(k == KC - 1))
                ms = a1.tile([1, SLAB], F32, name=f"{y_name}_ms", tag="ln_ms")
                nc.scalar.activation(ms[:], st[0:1, :], AF.Square, scale=1.0 / CHID)
                var = a1.tile([1, SLAB], F32, name=f"{y_name}_var", tag="ln_var")
                nc.vector.scalar_tensor_tensor(var[:], st[32:33, :], 1.0 / CHID, ms[:],
                                               op0=ALU.mult, op1=ALU.subtract)
                std = a1.tile([1, SLAB], F32, name=f"{y_name}_std", tag="ln_std")
                nc.scalar.activation(std[:], var[:], AF.Sqrt, bias=epsc_t[0:1, 0:1])
                r = a1.tile([1, SLAB], F32, name=f"{y_name}_r", tag="ln_r")
                nc.vector.reciprocal(r[:], std[:])
                rbf = a1.tile([1, SLAB], BF16, name=f"{y_name}_rbf", tag="ln_rbf")
                nc.scalar.activation(rbf[:], r[:], AF.Copy)
                mrbf = a1.tile([1, SLAB], BF16, name=f"{y_name}_mrbf", tag="ln_mrbf")
                nc.vector.scalar_tensor_tensor(mrbf[:], st[0:1, :], 1.0 / CHID, r[:],
                                               op0=ALU.mult, op1=ALU.mult)
                rb = ps2.tile([P, SLAB], F32, name=f"{y_name}_rb", tag="mmout")
                nc.tensor.matmul(rb[:], onesrow_t[:], rbf[:])
                mrb = ps2.tile([P, SLAB], F32, name=f"{y_name}_mrb", tag="mmout")
                nc.tensor.matmul(mrb[:], onesrow_t[:], mrbf[:])
                y = ypool.tile([P, KC * SLAB], BF16, name=y_name, tag=y_name)
                tmp = a1.tile([P, SLAB], F32, name=f"{y_name}_tmp", tag="ln_tmp")
                for k in range(KC):
                    sl = slice(k * SLAB, (k + 1) * SLAB)
                    nc.vector.tensor_mul(tmp[:], h_t[:, sl], rb[:])
                    nc.vector.tensor_sub(y[:, sl], tmp[:], mrb[:])
                return y

            # ---------------------------------------------------------------
            def emit_block(h_t, w, out_dtype=F32):
                """One transformer block on a 512-token slab.
                h_t: [128, KC*512] f32 (residual stream, [c,t] layout).
                Returns x2 tile (same layout)."""
                # ---- LN1 -> y ----
                if EN_LN:
                    y = emit_ln(h_t, "y_bf", a2)
                else:
                    y = a2.tile([P, KC * SLAB], BF16, name="y_bf", tag="y_bf")
                    for k in range(KC):
                        sl = slice(k * SLAB, (k + 1) * SLAB)
                        nc.scalar.activation(y[:, sl], h_t[:, sl], AF.Copy)

                if not EN_ATTN:
                    x1 = a1.tile([P, KC * SLAB], F32, name="x1", tag="x1")
                    for k in range(KC):
                        sl = slice(k * SLAB, (k + 1) * SLAB)
                        nc.vector.tensor_scalar(x1[:, sl], h_t[:, sl], 1.0, None,
                                                op0=ALU.mult)
                    return _finish_block(x1, h_t, w, out_dtype)
                # ---- qkv: Q^T,K^T (weight-stationary) ----
                # cols [0, 4096): m-chunk evictions (head pair 2m/2m+1 stacked);
                # cols [4096, 8192): bottom halves (odd heads) DMA-moved to
                # partitions 0-63 -- matmul operands at partition base 64 crash
                # the PE when bases alternate across matmuls, so every S-matmul
                # operand must start at partition 0.
                QK2 = 2 * KC * SLAB
                qk = a2.tile([P, 2 * QK2], BF16, name="qk", tag="qk", bufs=1)
                for m in range(2 * KC):
                    ps = ps2.tile([P, SLAB], F32, name=f"qkps{m}", tag="mmout")
                    for k in range(KC):
                        nc.tensor.matmul(ps[:], w["wqkv"][k][:, m * P:(m + 1) * P],
                                         y[:, k * SLAB:(k + 1) * SLAB],
                                         start=(k == 0), stop=(k == KC - 1))
                    nc.scalar.activation(qk[:, m * SLAB:(m + 1) * SLAB], ps[:],
                                         AF.Identity, bias=w["bqk"][:, m:m + 1])
                    nc.sync.dma_start(out=qk[0:64, QK2 + m * SLAB:QK2 + (m + 1) * SLAB],
                                      in_=qk[64:128, m * SLAB:(m + 1) * SLAB])

                # ---- V (activation-stationary -> [t, d]) ----
                v = a2.tile([P, KC * SLAB], BF16, name="v", tag="v")
                for tch in range(TC):
                    ps = ps2.tile([P, CHID], F32, name=f"vps{tch}", tag="mmout")
                    for k in range(KC):
                        nc.tensor.matmul(ps[:], y[:, k * SLAB + tch * P: k * SLAB + (tch + 1) * P],
                                         w["wqkv"][k][:, 2 * CHID:3 * CHID],
                                         start=(k == 0), stop=False)
                    nc.tensor.matmul(ps[:], onesrow_t[:], w["bvrow"][:],
                                     start=False, stop=True)
                    nc.scalar.activation(v[:, tch * SLAB:(tch + 1) * SLAB], ps[:], AF.Copy)

                # ---- windowed attention, per 128-token chunk ----
                ot = a2.tile([P, KC * SLAB], BF16, name="ot", tag="ot")
                for tch in range(TC):
                    if ATTN_DEPTH < 2:
                        nc.scalar.activation(ot[:, tch * SLAB:(tch + 1) * SLAB],
                                             v[:, tch * SLAB:(tch + 1) * SLAB], AF.Copy)
                        continue
                    s_ps = ps1.tile([P, NH * P], F32, name=f"s{tch}", tag="S")
                    for h in range(NH):
                        hoff = 0 if h % 2 == 0 else QK2
                        qsl = qk[0:64, hoff + (h // 2) * SLAB + tch * P:
                                 hoff + (h // 2) * SLAB + (tch + 1) * P]
                        ksl = qk[0:64, hoff + (KC + h // 2) * SLAB + tch * P:
                                 hoff + (KC + h // 2) * SLAB + (tch + 1) * P]
                        nc.tensor.matmul(s_ps[:, h * P:(h + 1) * P], qsl, ksl)
                    e_t = a2.tile([P, NH * P], F32, name=f"e{tch}", tag="E")
                    for h in range(NH):
                        nc.scalar.activation(e_t[:, h * P:(h + 1) * P],
                                             s_ps[:, h * P:(h + 1) * P], AF.Exp,
                                             scale=float(HD) ** -0.5)
                    em_t = e_t
                    sums = a1.tile([P, NH], F32, name=f"sums{tch}", tag="sums")
                    for h in range(NH):
                        nc.vector.scalar_tensor_tensor(em_t[:, h * P:(h + 1) * P],
                                                       e_t[:, h * P:(h + 1) * P], 1.0,
                                                       mask_t[:],
                                                       op0=ALU.mult, op1=ALU.mult,
                                                       accum_out=sums[:, h:h + 1])
                    rec = a1.tile([P, NH], F32, name=f"rec{tch}", tag="rec")
                    nc.vector.reciprocal(rec[:], sums[:])
                    for h in range(NH):
                        nc.vector.tensor_scalar(em_t[:, h * P:(h + 1) * P],
                                                em_t[:, h * P:(h + 1) * P],
                                                rec[:, h:h + 1], None, op0=ALU.mult)
                    if ATTN_DEPTH < 3:
                        nc.scalar.activation(ot[:, tch * SLAB:(tch + 1) * SLAB],
                                             v[:, tch * SLAB:(tch + 1) * SLAB], AF.Copy)
                        continue
                    pt_ps = ps1.tile([P, NH * P], F32, name=f"ptps{tch}", tag="PT")
                    pt = a2.tile([P, NH * P], BF16, name=f"pt{tch}", tag="PTS")
                    for h in range(NH):
                        nc.tensor.transpose(pt_ps[:, h * P:(h + 1) * P],
                                            em_t[:, h * P:(h + 1) * P], ident_t[:])
                        nc.scalar.activation(pt[:, h * P:(h + 1) * P],
                                             pt_ps[:, h * P:(h + 1) * P], AF.Copy)
                    if ATTN_DEPTH < 4:
                        nc.scalar.activation(ot[:, tch * SLAB:(tch + 1) * SLAB],
                                             v[:, tch * SLAB:(tch + 1) * SLAB], AF.Copy)
                        continue
                    o_ps = ps1.tile([P, SLAB], F32, name=f"ops{tch}", tag="OT")
                    for h in range(NH):
                        nc.tensor.matmul(o_ps[64 * (h % 2):64 * (h % 2) + 64,
                                              (h // 2) * P:(h // 2 + 1) * P],
                                         v[:, tch * SLAB + 64 * h:tch * SLAB + 64 * h + 64],
                                         pt[:, h * P:(h + 1) * P])
                    nc.scalar.activation(ot[:, tch * SLAB:(tch + 1) * SLAB], o_ps[:], AF.Copy)

                # ---- proj + residual ----
                x1 = a1.tile([P, KC * SLAB], F32, name="x1", tag="x1")
                ot_r = ot[:].rearrange("p (t j q) -> p t j q", t=TC, j=KC, q=P)
                for m in range(KC):
                    ps = ps2.tile([P, SLAB], F32, name=f"pps{m}", tag="mmout")
                    for k in range(KC):
                        nc.tensor.matmul(ps[:], w["wpw"][k][:, m * P:(m + 1) * P],
                                         ot_r[:, :, k, :],
                                         start=(k == 0), stop=(k == KC - 1))
                    nc.vector.scalar_tensor_tensor(x1[:, m * SLAB:(m + 1) * SLAB], ps[:],
                                                   w["bpb"][:, m:m + 1],
                                                   h_t[:, m * SLAB:(m + 1) * SLAB],
                                                   op0=ALU.add, op1=ALU.add)
                return _finish_block(x1, h_t, w, out_dtype)

            def _finish_block(x1, h_t, w, out_dtype):
                # ---- LN2 -> z ----
                if EN_LN:
                    z = emit_ln(x1, "z_bf", a1)
                else:
                    z = a1.tile([P, KC * SLAB], BF16, name="z_bf", tag="z_bf")
                    for k in range(KC):
                        sl = slice(k * SLAB, (k + 1) * SLAB)
                        nc.scalar.activation(z[:, sl], x1[:, sl], AF.Copy)
                if not EN_MLP:
                    x2 = a2.tile([P, KC * SLAB], out_dtype, name="x2", tag="x2")
                    for k in range(KC):
                        sl = slice(k * SLAB, (k + 1) * SLAB)
                        nc.vector.tensor_scalar(x2[:, sl], x1[:, sl], 1.0, None,
                                                op0=ALU.mult)
                    return x2
                # ---- MLP ----
                g = a1.tile([P, (CH4 // P) * SLAB], BF16, name="g", tag="g")
                for m in range(CH4 // P):
                    ps = ps2.tile([P, SLAB], F32, name=f"m1ps{m}", tag="mmout")
                    for k in range(KC):
                        nc.tensor.matmul(ps[:], w["wm1"][k][:, m * P:(m + 1) * P],
                                         z[:, k * SLAB:(k + 1) * SLAB],
                                         start=(k == 0), stop=(k == KC - 1))
                    nc.scalar.activation(g[:, m * SLAB:(m + 1) * SLAB], ps[:], GELU_FN,
                                         bias=w["bm1"][:, m:m + 1])
                x2 = a2.tile([P, KC * SLAB], out_dtype, name="x2", tag="x2")
                for m in range(KC):
                    ps = ps2.tile([P, SLAB], F32, name=f"m2ps{m}", tag="mmout")
                    for k in range(CH4 // P):
                        nc.tensor.matmul(ps[:], w["wm2"][k][:, m * P:(m + 1) * P],
                                         g[:, k * SLAB:(k + 1) * SLAB],
                                         start=(k == 0), stop=(k == CH4 // P - 1))
                    nc.vector.scalar_tensor_tensor(x2[:, m * SLAB:(m + 1) * SLAB], ps[:],
                                                   w["bm2"][:, m:m + 1],
                                                   x1[:, m * SLAB:(m + 1) * SLAB],
                                                   op0=ALU.add, op1=ALU.add)
                return x2

            # ===============================================================
            # phase 1: x -> ff1 -> block A -> h1 scratch
            # ===============================================================
            t_w = load_block_weights("a")
            for b in range(cfg.n1):
                t0 = b * SLAB
                xin = a1.tile([P, TC * CIN], F32, name=f"xin{b}", tag="xin")
                for i in range(TC):
                    nc.sync.dma_start(out=xin[:, i * CIN:(i + 1) * CIN],
                                      in_=x_in[t0 + i * P:t0 + (i + 1) * P, :])
                xT = a1.tile([P, (CIN // P) * SLAB], FF_DT, name=f"xT{b}", tag="xT")
                for j in range(CIN // P):
                    tps = ps2.tile([P, SLAB], F32, name=f"tps{b}_{j}", tag="mmout")
                    for i in range(TC):
                        nc.tensor.transpose(tps[:, i * P:(i + 1) * P],
                                            xin[:, i * CIN + j * P:i * CIN + (j + 1) * P],
                                            ident_t[:])
                    nc.scalar.activation(xT[:, j * SLAB:(j + 1) * SLAB], tps[:], AF.Copy)
                h_t = a2.tile([P, KC * SLAB], F32, name=f"h{b}", tag="h")
                for m in range(KC):
                    ps = ps2.tile([P, SLAB], F32, name=f"f1ps{b}_{m}", tag="mmout")
                    for j in range(CIN // P):
                        nc.tensor.matmul(ps[:],
                                         wff1_t[:, j * CHID + m * P:j * CHID + (m + 1) * P],
                                         xT[:, j * SLAB:(j + 1) * SLAB],
                                         start=(j == 0), stop=(j == CIN // P - 1))
                    nc.scalar.activation(h_t[:, m * SLAB:(m + 1) * SLAB], ps[:], GELU_FN,
                                         bias=bff1_t[:, m:m + 1])
                x2 = emit_block(h_t, t_w)
                for k in range(KC):
                    nc.sync.dma_start(out=h1_t[k, :, t0:t0 + SLAB],
                                      in_=x2[:, k * SLAB:(k + 1) * SLAB])

            # ===============================================================
            # phase 2: h1 -> block B -> ff2 -> out
            # ===============================================================
            t_w = load_block_weights("b")
            for b in range(cfg.n2):
                c0 = b * SLAB + HALO - WS // 2  # slab origin in scratch coords
                h_t = a2.tile([P, KC * SLAB], F32, name=f"hb{b}", tag="h")
                for k in range(KC):
                    nc.sync.dma_start(out=h_t[:, k * SLAB:(k + 1) * SLAB],
                                      in_=h1_t[k, :, c0:c0 + SLAB])
                x2 = emit_block(h_t, t_w, out_dtype=FF_DT)
                # ff2 (fp32r, activation-stationary -> [t, c] directly)
                o_t = a1.tile([P, TC * COUT], F32, name=f"o{b}", tag="o")
                out_base = b * SLAB - WS // 2  # first out row this slab covers
                for tch in range(TC):
                    r0 = out_base + tch * P  # out rows [r0, r0+128)
                    lo, hi = max(r0, 0), min(r0 + P, cfg.t_out)
                    if lo >= hi:
                        continue
                    ps = ps2.tile([P, COUT], F32, name=f"f2ps{b}_{tch}", tag="mmout")
                    for k in range(KC):
                        nc.tensor.matmul(ps[:],
                                         x2[:, k * SLAB + tch * P:k * SLAB + (tch + 1) * P],
                                         wff2_t[:, k * COUT:(k + 1) * COUT],
                                         start=(k == 0), stop=False)
                    nc.tensor.matmul(ps[:], onesrowf_t[:],
                                     bff2_t[:], start=False, stop=True)
                    nc.scalar.activation(o_t[:, tch * COUT:(tch + 1) * COUT], ps[:], AF.Copy)
                    nc.sync.dma_start(out=out[lo:hi, :],
                                      in_=o_t[lo - r0:hi - r0, tch * COUT:(tch + 1) * COUT])

    nc.compile()
    return nc


# ---------------------------------------------------------------------------
# host-side input preparation
# ---------------------------------------------------------------------------


def _sigma(W, u):
    W = np.asarray(W, np.float32)
    u = np.asarray(u, np.float32)
    v = W @ u
    v = v / (np.linalg.norm(v) + 1e-12)
    u2 = v @ W
    u2 = u2 / (np.linalg.norm(u2) + 1e-12)
    return float(v @ W @ u2)


def prep_weights(inputs):
    """Host-side: fold LN affine + spectral norm into weights; tile/cast."""
    f32 = np.float32
    d = {}
    w1 = np.asarray(inputs["ff1_w"], f32)
    d["wff1"] = np.ascontiguousarray(w1.reshape(CIN // P, P, CHID))
    d["bff1"] = np.ascontiguousarray(np.asarray(inputs["ff1_b"], f32).reshape(KC, P).T)

    sig = _sigma(inputs["ff2_w"], inputs["ff2_u"])
    w2 = np.asarray(inputs["ff2_w"], f32) / sig
    d["wff2"] = np.ascontiguousarray(w2.reshape(KC, P, COUT))
    d["bff2row"] = np.asarray(inputs["ff2_b"], f32).reshape(1, COUT)

    for p in ("a", "b"):
        g1 = np.asarray(inputs[f"{p}_ln1g"], f32)
        b1 = np.asarray(inputs[f"{p}_ln1b"], f32)
        qkvw = np.asarray(inputs[f"{p}_qkvw"], f32)
        qkvb = np.asarray(inputs[f"{p}_qkvb"], f32) + b1 @ qkvw
        wg = g1[:, None] * qkvw  # [512, 1536]
        d[f"{p}_wqkv"] = _to_bf16(wg.reshape(KC, P, 3 * CHID))
        d[f"{p}_bqk"] = np.ascontiguousarray(qkvb[:2 * CHID].reshape(2 * KC, P).T)
        d[f"{p}_bvrow"] = _to_bf16(qkvb[2 * CHID:].reshape(1, CHID))
        pw = np.asarray(inputs[f"{p}_pw"], f32)
        d[f"{p}_wpw"] = _to_bf16(pw.reshape(KC, P, CHID))
        d[f"{p}_bpb"] = np.ascontiguousarray(np.asarray(inputs[f"{p}_pb"], f32).reshape(KC, P).T)
        g2 = np.asarray(inputs[f"{p}_ln2g"], f32)
        b2 = np.asarray(inputs[f"{p}_ln2b"], f32)
        m1w = np.asarray(inputs[f"{p}_m1w"], f32)
        m1b = np.asarray(inputs[f"{p}_m1b"], f32) + b2 @ m1w
        d[f"{p}_wm1"] = _to_bf16((g2[:, None] * m1w).reshape(KC, P, CH4))
        d[f"{p}_bm1"] = np.ascontiguousarray(m1b.reshape(CH4 // P, P).T)
        m2w = np.asarray(inputs[f"{p}_m2w"], f32)
        d[f"{p}_wm2"] = _to_bf16(m2w.reshape(CH4 // P, P, CHID))
        d[f"{p}_bm2"] = np.ascontiguousarray(np.asarray(inputs[f"{p}_m2b"], f32).reshape(KC, P).T)

    d["ident32"] = np.eye(P, dtype=f32)
    m = np.zeros((P, P), f32)
    for wdw in range(P // WS):
        m[wdw * WS:(wdw + 1) * WS, wdw * WS:(wdw + 1) * WS] = 1.0
    d["maskbd"] = m
    d["ones_col_bf"] = _to_bf16(np.ones((P, 1), f32))
    d["ones_row_bf"] = _to_bf16(np.ones((1, P), f32))
    d["ones_row_f32"] = np.ones((1, P), f32)
    d["eps_t"] = np.full((1, 1), 1e-5, f32)
    return d


def _to_bf16(a):
    import ml_dtypes
    return np.ascontiguousarray(np.asarray(a, np.float32)).astype(ml_dtypes.bfloat16)


def make_in_maps(x, wd, cfg: Cfg):
    """x: [B, N, CIN]. Returns per-core input maps."""
    maps = []
    cores_per_batch = max(1, cfg.n_cores // x.shape[0])
    for c in range(cfg.n_cores):
        beta = c // cores_per_batch
        eta = c % cores_per_batch
        start = eta * cfg.t_out - HALO
        idx = (start + np.arange(cfg.t_in)) % x.shape[1]
        m = {"x_in": np.ascontiguousarray(x[beta, idx], np.float32)}
        m.update(wd)
        maps.append(m)
    return maps


_PROG = {}


def _get_prog(cfg: Cfg):
    key = (cfg.t_out, cfg.n_cores)
    if key not in _PROG:
        _PROG[key] = build_program(cfg)
    return _PROG[key]


def kernel(**inputs) -> np.ndarray:
    x = np.asarray(inputs["x"], np.float32)
    Bx, Nx = x.shape[0], x.shape[1]
    n_cores = 8
    cores_per_batch = n_cores // Bx
    cfg = Cfg(t_out=Nx // cores_per_batch, n_cores=n_cores)
    nc = _get_prog(cfg)
    wd = prep_weights(inputs)
    in_maps = make_in_maps(x, wd, cfg)
    res = run_bass_kernel_spmd(nc, in_maps, core_ids=list(range(n_cores)))
    out = np.empty((Bx, Nx, COUT), np.float32)
    for c in range(n_cores):
        beta = c // cores_per_batch
        eta = c % cores_per_batch
        out[beta, eta * cfg.t_out:(eta + 1) * cfg.t_out] = res.results[c]["out"]
    return out

